# revision 1
# baseline (speedup 1.0000x reference)
"""DCRNN seq2seq (encoder/decoder DCGRU, K=3 Chebyshev diffusion) on 8 NeuronCores.

Sharding: data-parallel over batch (8 batch elements per core); weights and the
200x200 support replicated; no collectives.

Per-core layout strategy:
  - Diffusion (contract over nodes) consumes node-major tiles:
      lhsT = Z^T (node m on partitions, feat free), rhs = [S1 | S2]
      -> psum (feat partitions, 2*200 free), i.e. f-major per-b output.
    Chebyshev x2 uses S2 = 2 S @ S - I precomputed host-side, so k=1,2 are
    independent matmuls sharing one rhs tile. For layers >= 1 the gates input
    [h_{l-1} | h_l] is a contiguous 128-wide window of the slotted node-major
    state tile, so one matmul diffuses both halves; the candidate only needs
    the extra r*h diffusion (64-wide) and reuses the gates' x half.
  - Gate/candidate matmuls (contract features) use the f-major tiles:
      lhsT = W k-slices (feat partitions, out free), rhs = Xk (feat, n per b)
      -> psum (out partitions, n free) accumulated over k-slices.
  - sigmoid/tanh on ScalarE read psum directly with per-partition bias fused;
    GRU elementwise update on VectorE in bf16 (2x mode).
  - h (f-major bf16) is transposed into the node-major slot tile with DMA xbar
    transposes right after each update.
  - Decoder projection: lhsT = [h3; ones] slices, rhs = [proj_W; proj_b]
    (bias via augmented contraction row); decoder layer-0 k=0 x-term is
    algebraically fused: proj @ W0x_k0 = h3 @ (Wp @ W0x_k0), precomputed host-
    side, so only the diffused (k=1,2) x-terms need the materialized proj.

All matmul operands bf16 (fp32 psum accumulate); measured end-to-end rel l2
error vs the fp32 reference ~2e-3.
"""

import numpy as np
import ml_dtypes

import concourse.bass as bass
import concourse.tile as tile
from concourse import bacc, mybir
from concourse.bass_utils import run_bass_kernel_spmd

BF = ml_dtypes.bfloat16
F32 = np.float32

N = 200
U = 64
L = 4
T = 12
B = 64
NCORES = 8
BL = B // NCORES
M0, M1 = 128, 72
PADN = 256

dt = mybir.dt
AF = mybir.ActivationFunctionType

_CACHE = {}


def _build(enc_T=T, dec_T=T):
    nc = bacc.Bacc()

    d = {}

    def din(name, shape, dtype=dt.bfloat16):
        d[name] = nc.dram_tensor(name, shape, dtype, kind='ExternalInput')

    din('SS0', [M0, 400])
    din('SS1', [M1, 400])
    din('Wp', [U + 1, 200])
    for p in ('e', 'd'):
        din(p + 'g0x', [200, 3, 128])
        din(p + 'g0h', [64, 3, 128])
        din(p + 'c0x', [200, 3, 64])
        din(p + 'c0h', [64, 3, 64])
        din(p + 'gL', [128, 3, 3, 128])
        din(p + 'gLh', [64, 3, 128])
        din(p + 'cLk0x', [64, 3, 64])
        din(p + 'cLh', [64, 3, 64])
        din(p + 'cLx', [64, 3, 2, 64])
        din(p + 'cLrh', [64, 3, 2, 64])
        din(p + 'bg', [128, 4], dt.float32)
        din(p + 'bc', [64, 4], dt.float32)
    din('dWfg', [64, 128])
    din('dWfc', [64, 64])
    din('xTe', [enc_T, 2, M0, BL, 200])
    din('xfme', [enc_T, 2, M0, BL, 200])
    d['onm'] = nc.dram_tensor('onm', [max(dec_T, 1), 200, BL, 200], dt.float32,
                              kind='ExternalOutput')

    with tile.TileContext(nc) as tc:
        with (
            tc.tile_pool(name='const', bufs=1) as cp,
            tc.tile_pool(name='state', bufs=1) as sp,
            tc.tile_pool(name='work', bufs=2) as wp,
            tc.tile_pool(name='xin', bufs=2) as xp,
            tc.tile_pool(name='dps', bufs=3, space='PSUM') as diffps,
            tc.tile_pool(name='ops', bufs=2, space='PSUM') as outps,
        ):
            # ---- load constants / weights ----
            CT = {}
            for name, t_ in d.items():
                if name in ('onm', 'xTe', 'xfme'):
                    continue
                shape = list(t_.shape)
                if shape[0] == 200:  # split node-feature-major weights
                    CT[name + '@a'] = cp.tile([M0] + shape[1:], t_.dtype, name='t' + name + 'a')
                    CT[name + '@b'] = cp.tile([M1] + shape[1:], t_.dtype, name='t' + name + 'b')
                    nc.sync.dma_start(out=CT[name + '@a'], in_=t_[0:M0])
                    nc.sync.dma_start(out=CT[name + '@b'], in_=t_[M0:200])
                else:
                    CT[name] = cp.tile(shape, t_.dtype, name='t' + name)
                    nc.sync.dma_start(out=CT[name], in_=t_[:])
            SS = [CT['SS0'], CT['SS1']]
            Wp = CT['Wp']

            # ---- state ----
            HFM = [sp.tile([64, BL, PADN], dt.bfloat16, name=f'HFM{i}') for i in range(3)]
            HFM.append(sp.tile([65, BL, PADN], dt.bfloat16, name='HFM3'))
            # node-major slot tile: slots [h0 h1 h2 h3 rh]
            HT = [sp.tile([M0, BL, 5, 64], dt.bfloat16, name=f'HT{i}') for i in range(2)]

            for h in HFM:
                nc.vector.memset(h[:], 0.0)
            nc.vector.memset(HFM[3][64:65, :, :], 1.0)
            for t_ in HT:
                nc.vector.memset(t_[:], 0.0)

            def evac(i, dst, src):
                if i % 2 == 0:
                    nc.scalar.copy(dst, src)
                else:
                    nc.vector.tensor_copy(dst, src)

            def diffuse(rows, lhs, dst_ap, i):
                """psum[0:rows, 0:400] = [lhs.T @ S1 | lhs.T @ S2], evacuated
                (bf16) to dst_ap. lhs = per-m-chunk lhsT APs."""
                ps = diffps.tile([M0, 400], dt.float32, name='dps', tag='dps')
                nc.tensor.matmul(ps[0:rows, :], lhs[0], SS[0][:], start=True, stop=False)
                nc.tensor.matmul(ps[0:rows, :], lhs[1], SS[1][:], start=False, stop=True)
                evac(i, dst_ap, ps[0:rows, :])

            def transpose_to_HT(src, slot):
                for b in range(BL):
                    nc.sync.dma_start_transpose(
                        HT[0][:, b, slot, :], src[0:64, b, 0:128])
                    nc.sync.dma_start_transpose(
                        HT[1][:, b, slot, :], src[0:64, b, 128:256])

            def gru_tail(l, UFM, CFM, TMP):
                h = HFM[l][0:64, :, 0:200]
                c = CFM[0:64, :, 0:200]
                u_ = UFM[0:64, :, 0:200]
                t_ = TMP[0:64, :, 0:200]
                nc.vector.tensor_sub(t_, h, c)
                nc.vector.tensor_mul(t_, u_, t_)
                nc.vector.tensor_add(h, c, t_)
                transpose_to_HT(HFM[l], l)

            def sig_tanh(psum, bias, RFM, UFM, half):
                s = slice(half * 4, half * 4 + 4)
                if UFM is None:
                    nc.scalar.activation(RFM[0:64, s, 0:200], psum[0:64, :, 0:200],
                                         AF.Tanh, bias=bias, scale=1.0)
                else:
                    nc.scalar.activation(RFM[0:64, s, 0:200], psum[0:64, :, 0:200],
                                         AF.Sigmoid, bias=bias[0:64], scale=1.0)
                    nc.scalar.activation(UFM[0:64, s, 0:200], psum[64:128, :, 0:200],
                                         AF.Sigmoid, bias=bias[64:128], scale=1.0)

            def cell_upper(p, l):
                gL, gLh = CT[p + 'gL'], CT[p + 'gLh']
                cLk0x, cLh = CT[p + 'cLk0x'], CT[p + 'cLh']
                cLx, cLrh = CT[p + 'cLx'], CT[p + 'cLrh']
                # -- gates: diffuse [h_{l-1} | h_l] (contiguous slot window) --
                Xg = wp.tile([M0, BL, 400], dt.bfloat16, name='Xg', tag='Xg')
                for b in range(BL):
                    diffuse(128, [HT[0][:, b, l - 1:l + 1, :],
                                  HT[1][0:M1, b, l - 1:l + 1, :]], Xg[:, b, :], b)
                RFM = wp.tile([64, BL, PADN], dt.bfloat16, name='RFM', tag='RFM')
                UFM = wp.tile([64, BL, PADN], dt.bfloat16, name='UFM', tag='UFM')
                for half in range(2):
                    gps = outps.tile([M0, 4, 256], dt.float32, name='ops', tag='ops')
                    for bb in range(4):
                        b = half * 4 + bb
                        o = gps[:, bb, 0:200]
                        nc.tensor.matmul(o, gL[0:64, l - 1, 0, :], HFM[l - 1][0:64, b, 0:200], start=True, stop=False)
                        nc.tensor.matmul(o, gLh[:, l - 1, :], HFM[l][0:64, b, 0:200], start=False, stop=False)
                        nc.tensor.matmul(o, gL[:, l - 1, 1, :], Xg[:, b, 0:200], start=False, stop=False)
                        nc.tensor.matmul(o, gL[:, l - 1, 2, :], Xg[:, b, 200:400], start=False, stop=True)
                    sig_tanh(gps, CT[p + 'bg'][:, l:l + 1], RFM, UFM, half)
                # -- r*h -> rh slot --
                RHFM = wp.tile([64, BL, PADN], dt.bfloat16, name='RHFM', tag='RHFM')
                nc.vector.tensor_mul(RHFM[0:64, :, 0:200],
                                     RFM[0:64, :, 0:200],
                                     HFM[l][0:64, :, 0:200])
                nc.vector.memset(RHFM[0:64, :, 200:PADN], 0.0)
                transpose_to_HT(RHFM, 4)
                # -- candidate: diffuse rh only; x half reuses Xg rows 0:64 --
                Xrh = wp.tile([64, BL, 400], dt.bfloat16, name='Xrh', tag='Xrh')
                for b in range(BL):
                    diffuse(64, [HT[0][:, b, 4, :],
                                 HT[1][0:M1, b, 4, :]], Xrh[0:64, b, :], b)
                CFM = wp.tile([64, BL, PADN], dt.bfloat16, name='CFM', tag='CFM')
                for half in range(2):
                    cps = outps.tile([M0, 4, 256], dt.float32, name='ops', tag='ops')
                    for bb in range(4):
                        b = half * 4 + bb
                        o = cps[0:64, bb, 0:200]
                        nc.tensor.matmul(o, cLk0x[:, l - 1, :], HFM[l - 1][0:64, b, 0:200], start=True, stop=False)
                        nc.tensor.matmul(o, cLh[:, l - 1, :], RHFM[0:64, b, 0:200], start=False, stop=False)
                        nc.tensor.matmul(o, cLx[:, l - 1, 0, :], Xg[0:64, b, 0:200], start=False, stop=False)
                        nc.tensor.matmul(o, cLrh[:, l - 1, 0, :], Xrh[0:64, b, 0:200], start=False, stop=False)
                        nc.tensor.matmul(o, cLx[:, l - 1, 1, :], Xg[0:64, b, 200:400], start=False, stop=False)
                        nc.tensor.matmul(o, cLrh[:, l - 1, 1, :], Xrh[0:64, b, 200:400], start=False, stop=True)
                    sig_tanh(cps, CT[p + 'bc'][:, l:l + 1], CFM, None, half)
                TMP = wp.tile([64, BL, PADN], dt.bfloat16, name='TMP', tag='TMP')
                gru_tail(l, UFM, CFM, TMP)

            def cell0(p, x_terms, x0T, xfm):
                enc = (p == 'e')
                g0xa, g0xb, g0h = CT[p + 'g0x@a'], CT[p + 'g0x@b'], CT[p + 'g0h']
                c0xa, c0xb, c0h = CT[p + 'c0x@a'], CT[p + 'c0x@b'], CT[p + 'c0h']
                Xga = wp.tile([M0, BL, 400], dt.bfloat16, name='Xg', tag='Xg')
                Xgb = wp.tile([M1, BL, 400], dt.bfloat16, name='Xgb', tag='Xgb')
                Xgh = wp.tile([64, BL, 400], dt.bfloat16, name='Xgh', tag='Xgh')
                if x_terms:
                    for b in range(BL):
                        diffuse(128, [x0T[0][:, b, 0:128],
                                      x0T[1][0:M1, b, 0:128]], Xga[:, b, :], b)
                    for b in range(BL):
                        diffuse(M1, [x0T[0][:, b, 128:200],
                                     x0T[1][0:M1, b, 128:200]], Xgb[0:M1, b, :], b)
                for b in range(BL):
                    diffuse(64, [HT[0][:, b, 0, :],
                                 HT[1][0:M1, b, 0, :]], Xgh[0:64, b, :], b)
                RFM = wp.tile([64, BL, PADN], dt.bfloat16, name='RFM', tag='RFM')
                UFM = wp.tile([64, BL, PADN], dt.bfloat16, name='UFM', tag='UFM')
                for half in range(2):
                    gps = outps.tile([M0, 4, 256], dt.float32, name='ops', tag='ops')
                    for bb in range(4):
                        b = half * 4 + bb
                        o = gps[:, bb, 0:200]
                        first = True
                        if x_terms:
                            if enc:
                                nc.tensor.matmul(o, g0xa[:, 0, :], xfm[0][:, b, 0:200], start=True, stop=False)
                                nc.tensor.matmul(o, g0xb[0:M1, 0, :], xfm[1][0:M1, b, 0:200], start=False, stop=False)
                            else:
                                nc.tensor.matmul(o, CT['dWfg'][:], HFM[3][0:64, b, 0:200], start=True, stop=False)
                            for k in (1, 2):
                                s = slice(200 * (k - 1), 200 * k)
                                nc.tensor.matmul(o, g0xa[:, k, :], Xga[:, b, s], start=False, stop=False)
                                nc.tensor.matmul(o, g0xb[0:M1, k, :], Xgb[0:M1, b, s], start=False, stop=False)
                            first = False
                        nc.tensor.matmul(o, g0h[:, 0, :], HFM[0][0:64, b, 0:200], start=first, stop=False)
                        nc.tensor.matmul(o, g0h[:, 1, :], Xgh[0:64, b, 0:200], start=False, stop=False)
                        nc.tensor.matmul(o, g0h[:, 2, :], Xgh[0:64, b, 200:400], start=False, stop=True)
                    sig_tanh(gps, CT[p + 'bg'][:, 0:1], RFM, UFM, half)
                RHFM = wp.tile([64, BL, PADN], dt.bfloat16, name='RHFM', tag='RHFM')
                nc.vector.tensor_mul(RHFM[0:64, :, 0:200],
                                     RFM[0:64, :, 0:200],
                                     HFM[0][0:64, :, 0:200])
                nc.vector.memset(RHFM[0:64, :, 200:PADN], 0.0)
                transpose_to_HT(RHFM, 4)
                Xch = wp.tile([64, BL, 400], dt.bfloat16, name='Xgh', tag='Xgh')
                for b in range(BL):
                    diffuse(64, [HT[0][:, b, 4, :],
                                 HT[1][0:M1, b, 4, :]], Xch[0:64, b, :], b)
                CFM = wp.tile([64, BL, PADN], dt.bfloat16, name='CFM', tag='CFM')
                for half in range(2):
                    cps = outps.tile([M0, 4, 256], dt.float32, name='ops', tag='ops')
                    for bb in range(4):
                        b = half * 4 + bb
                        o = cps[0:64, bb, 0:200]
                        first = True
                        if x_terms:
                            if enc:
                                nc.tensor.matmul(o, c0xa[:, 0, :], xfm[0][:, b, 0:200], start=True, stop=False)
                                nc.tensor.matmul(o, c0xb[0:M1, 0, :], xfm[1][0:M1, b, 0:200], start=False, stop=False)
                            else:
                                nc.tensor.matmul(o, CT['dWfc'][:], HFM[3][0:64, b, 0:200], start=True, stop=False)
                            for k in (1, 2):
                                s = slice(200 * (k - 1), 200 * k)
                                nc.tensor.matmul(o, c0xa[:, k, :], Xga[:, b, s], start=False, stop=False)
                                nc.tensor.matmul(o, c0xb[0:M1, k, :], Xgb[0:M1, b, s], start=False, stop=False)
                            first = False
                        nc.tensor.matmul(o, c0h[:, 0, :], RHFM[0:64, b, 0:200], start=first, stop=False)
                        nc.tensor.matmul(o, c0h[:, 1, :], Xch[0:64, b, 0:200], start=False, stop=False)
                        nc.tensor.matmul(o, c0h[:, 2, :], Xch[0:64, b, 200:400], start=False, stop=True)
                    sig_tanh(cps, CT[p + 'bc'][:, 0:1], CFM, None, half)
                TMP = wp.tile([64, BL, PADN], dt.bfloat16, name='TMP', tag='TMP')
                gru_tail(0, UFM, CFM, TMP)

            # ---- encoder ----
            for t in range(enc_T):
                x0T = [xp.tile([M0, BL, 200], dt.bfloat16, name='x0T0', tag='x0T0'),
                       xp.tile([M0, BL, 200], dt.bfloat16, name='x0T1', tag='x0T1')]
                nc.sync.dma_start(out=x0T[0], in_=d['xTe'][t, 0])
                nc.sync.dma_start(out=x0T[1][0:M1], in_=d['xTe'][t, 1, 0:M1])
                xfm = [xp.tile([M0, BL, 200], dt.bfloat16, name='xf0', tag='xf0'),
                       xp.tile([M0, BL, 200], dt.bfloat16, name='xf1', tag='xf1')]
                nc.sync.dma_start(out=xfm[0], in_=d['xfme'][t, 0])
                nc.sync.dma_start(out=xfm[1][0:M1], in_=d['xfme'][t, 1, 0:M1])
                cell0('e', True, x0T, xfm)
                for l in range(1, L):
                    cell_upper('e', l)

            # ---- decoder ----
            x0T = None
            for t in range(dec_T):
                cell0('d', t > 0, x0T, None)
                for l in range(1, L):
                    cell_upper('d', l)
                pT = [wp.tile([M0, BL, 200], dt.float32, name='pT0', tag='pT0'),
                      wp.tile([M0, BL, 200], dt.float32, name='pT1', tag='pT1')]
                for mc, M in ((0, M0), (1, M1)):
                    for half in range(2):
                        pps = outps.tile([M0, 4, 256], dt.float32, name='ops', tag='ops')
                        for bb in range(4):
                            b = half * 4 + bb
                            nc.tensor.matmul(
                                pps[0:M, bb, 0:200],
                                HFM[3][0:65, b, mc * 128:mc * 128 + M],
                                Wp[:], start=True, stop=True)
                        evac(half, pT[mc][0:M, half * 4:half * 4 + 4, :],
                             pps[0:M, :, 0:200])
                nc.sync.dma_start(out=d['onm'][t, 0:M0], in_=pT[0][:])
                nc.sync.dma_start(out=d['onm'][t, M0:200], in_=pT[1][0:M1])
                if t < dec_T - 1:
                    x0T = [xp.tile([M0, BL, 200], dt.bfloat16, name='x0T0', tag='x0T0'),
                           xp.tile([M0, BL, 200], dt.bfloat16, name='x0T1', tag='x0T1')]
                    nc.vector.tensor_copy(x0T[0][:], pT[0][:])
                    nc.vector.tensor_copy(x0T[1][0:M1], pT[1][0:M1])

    nc.compile()
    return nc


# --------------------------------------------------------------------------
# host-side prep
# --------------------------------------------------------------------------

def _prep_shared(inputs):
    def bfc(x):
        return np.ascontiguousarray(np.asarray(x).astype(BF))

    S = np.asarray(inputs['support'], np.float64)
    S2 = 2.0 * (S @ S) - np.eye(N)
    SS = np.concatenate([S.astype(F32), S2.astype(F32)], axis=1)
    out = {
        'SS0': bfc(SS[0:M0]),
        'SS1': bfc(SS[M0:200]),
        'Wp': bfc(np.concatenate(
            [np.asarray(inputs['proj_W'], F32),
             np.asarray(inputs['proj_b'], F32)[None, :]], axis=0)),
    }
    for p, pre in (('e', 'enc_'), ('d', 'dec_')):
        Wg0 = np.asarray(inputs[pre + 'Wg0'], F32).reshape(264, 3, 128)
        Wc0 = np.asarray(inputs[pre + 'Wc0'], F32).reshape(264, 3, 64)
        out[p + 'g0x'] = bfc(Wg0[0:200])
        out[p + 'g0h'] = bfc(Wg0[200:264])
        out[p + 'c0x'] = bfc(Wc0[0:200])
        out[p + 'c0h'] = bfc(Wc0[200:264])
        WgL = np.asarray(inputs[pre + 'Wg'], F32).reshape(3, 128, 3, 128)
        WcL = np.asarray(inputs[pre + 'Wc'], F32).reshape(3, 128, 3, 64)
        out[p + 'gL'] = bfc(WgL.transpose(1, 0, 2, 3))          # (c, l-1, k, o)
        out[p + 'gLh'] = bfc(WgL[:, 64:128, 0, :].transpose(1, 0, 2))
        out[p + 'cLk0x'] = bfc(WcL[:, 0:64, 0, :].transpose(1, 0, 2))
        out[p + 'cLh'] = bfc(WcL[:, 64:128, 0, :].transpose(1, 0, 2))
        out[p + 'cLx'] = bfc(WcL[:, 0:64, 1:3, :].transpose(1, 0, 2, 3))
        out[p + 'cLrh'] = bfc(WcL[:, 64:128, 1:3, :].transpose(1, 0, 2, 3))
        bg = np.zeros((128, 4), F32)
        bc = np.zeros((64, 4), F32)
        bg[:, 0] = np.asarray(inputs[pre + 'bg0'], F32)
        bc[:, 0] = np.asarray(inputs[pre + 'bc0'], F32)
        bgl = np.asarray(inputs[pre + 'bg'], F32)
        bcl = np.asarray(inputs[pre + 'bc'], F32)
        for l in range(1, 4):
            bg[:, l] = bgl[l - 1]
            bc[:, l] = bcl[l - 1]
        if p == 'd':
            pb = np.asarray(inputs['proj_b'], np.float64)
            bg[:, 0] += (pb @ Wg0[0:200, 0, :].astype(np.float64)).astype(F32)
            bc[:, 0] += (pb @ Wc0[0:200, 0, :].astype(np.float64)).astype(F32)
            Wpf = np.asarray(inputs['proj_W'], np.float64)
            out['dWfg'] = bfc((Wpf @ Wg0[0:200, 0, :].astype(np.float64)).astype(F32))
            out['dWfc'] = bfc((Wpf @ Wc0[0:200, 0, :].astype(np.float64)).astype(F32))
        out[p + 'bg'] = np.ascontiguousarray(bg)
        out[p + 'bc'] = np.ascontiguousarray(bc)
    return out


def _prep_core_x(x_core, enc_T):
    x = np.asarray(x_core, F32).reshape(BL, -1, N, 200)[:, :enc_T]
    xb = x.astype(BF)
    xTe = np.zeros((enc_T, 2, M0, BL, 200), BF)
    xfme = np.zeros((enc_T, 2, M0, BL, 200), BF)
    xt = xb.transpose(1, 2, 0, 3)  # (T, n, b, f)
    xTe[:, 0, :, :, :] = xt[:, 0:M0]
    xTe[:, 1, 0:M1, :, :] = xt[:, M0:200]
    xf = xb.transpose(1, 3, 0, 2)  # (T, f, b, n)
    xfme[:, 0, :, :, :] = xf[:, 0:M0]
    xfme[:, 1, 0:M1, :, :] = xf[:, M0:200]
    return xTe, xfme


def get_program(enc_T=T, dec_T=T):
    key = (enc_T, dec_T)
    if key not in _CACHE:
        _CACHE[key] = _build(enc_T, dec_T)
    return _CACHE[key]


def make_in_maps(inputs, enc_T=T):
    shared = _prep_shared(inputs)
    x = np.asarray(inputs['inputs'], F32)
    in_maps = []
    for c in range(NCORES):
        xTe, xfme = _prep_core_x(x[c * BL:(c + 1) * BL], enc_T)
        m = dict(shared)
        m['xTe'] = xTe
        m['xfme'] = xfme
        in_maps.append(m)
    return in_maps


def assemble_output(results, dec_T=T):
    out = np.empty((B, dec_T, N * 200), F32)
    for c in range(NCORES):
        onm = results[c]['onm']
        out[c * BL:(c + 1) * BL] = (
            onm[:dec_T].transpose(2, 0, 1, 3).reshape(BL, dec_T, N * 200))
    return out


def kernel(**inputs):
    nc = get_program()
    in_maps = make_in_maps(inputs)
    res = run_bass_kernel_spmd(nc, in_maps, list(range(NCORES))).results
    return assemble_output(res)

